# revision 19
# baseline (speedup 1.0000x reference)
"""Trainium2 Bass kernel for nn_AdvancedGraphNeuralNetwork (3-layer GCN + pool + MLP).

Contract: kernel(**inputs) takes FULL unsharded inputs (x [20000,256] f32,
edge_index [2,320000] i32, weights), returns FULL output [1,2] f32.

Strategy (8 NeuronCores, graph/data parallel):
  - Nodes sharded across cores by dst ownership (2500/core), padded to
    2560 = 20 chunks x 128, per-core permutation balancing in-edge counts.
  - Node state TRANSPOSED in SBUF: xT [feat, nodes].
  - Per layer: local linear u^T = W^T xT scaled by dinv (PE) -> unm bf16
    (node-major, exact self term) + unm_q fp8e4m3 (256B-pitch rows) ->
    AllGather -> u_full [20480, 256B] fp8 in every core's HBM.
  - Gather tables are fp8 with narrow elem_size (L0: 256B, L1: 128B,
    L2: 64B) over a 256B row pitch -> gather DMA bytes cut 2-8x vs bf16.
  - One-hot S matrices are built ON-CHIP (vector iota==dstslot compare)
    instead of streamed from HBM (saves ~10MB/layer of DMA).
  - PE accumulates agg^T += M^T @ S^T over K edge blocks in PSUM, self
    term added via unm @ I, then GraphNorm + Gelu as before.
  - Pooling: free-axis sum/max + tiny AllGather; classifier on every core.
"""
import os
import numpy as np
import ml_dtypes

import concourse.bass as bass
from concourse import bacc
import concourse.mybir as mybir
import concourse.tile as tile
from concourse import bass_utils
from concourse import library_config
from concourse._compat import exact_div

F32 = mybir.dt.float32
BF16 = mybir.dt.bfloat16
FP8 = mybir.dt.float8e4
I16 = mybir.dt.int16
EPS = 1e-5
NCORES = 8
P = 128
PITCH = 256               # gather-table row pitch in bytes (fp8 elems)

# layer dims: fin -> fout
DIMS = [256, 256, 128, 64]
NLAYERS = 3
ELEM = [256, 128, 64]     # fp8 elems (= bytes) gathered per row per layer

NP_FP8 = ml_dtypes.float8_e4m3


def _ceil_div(a, b):
    return (a + b - 1) // b


import contextlib


def _nullcm():
    return contextlib.nullcontext()


# ----------------------------------------------------------------------------
# host-side graph preprocessing
# ----------------------------------------------------------------------------

def preprocess(x, edge_index):
    N = x.shape[0]
    src = edge_index[0].astype(np.int64)
    dst = edge_index[1].astype(np.int64)
    NPC = N // NCORES                      # real nodes per core
    NCHUNK = _ceil_div(NPC, P)             # chunks per core
    PADN = NCHUNK * P                      # padded nodes per core

    deg = np.bincount(dst, minlength=N).astype(np.float64) + 1.0
    dinv = (deg ** -0.5).astype(np.float32)

    incnt = np.bincount(dst, minlength=N)

    # per-core permutation: greedy-balance nodes into NCHUNK chunks of <=P
    slot_of = np.full(N, -1, dtype=np.int64)
    perm = np.full((NCORES, PADN), -1, dtype=np.int64)
    for c in range(NCORES):
        nodes = np.arange(c * NPC, (c + 1) * NPC)
        order = nodes[np.argsort(-incnt[nodes], kind="stable")]
        loads = np.zeros(NCHUNK, dtype=np.int64)
        counts = np.zeros(NCHUNK, dtype=np.int64)
        for nd in order:
            open_m = np.where(counts < P)[0]
            m = open_m[np.argmin(loads[open_m])]
            s = m * P + counts[m]
            counts[m] += 1
            loads[m] += incnt[nd]
            slot_of[nd] = s
            perm[c, s] = nd

    # group edges by (dst core, dst chunk)
    e_core = dst // NPC
    e_slot = slot_of[dst]
    e_chunk = e_slot // P
    e_voff = e_slot % P

    # uniform K across all cores/chunks (single SPMD program)
    K = 1
    for c in range(NCORES):
        m_of_core = e_chunk[e_core == c]
        for m in range(NCHUNK):
            cnt = int((m_of_core == m).sum())
            K = max(K, _ceil_div(max(cnt, 1), P))
    NBLK = NCHUNK * K
    NIDX = NBLK * P

    # gather row index in the rank-concatenated permuted padded u_full table
    gather_row = (src // NPC) * PADN + slot_of[src]   # [E]

    per_core = []
    for c in range(NCORES):
        sel = np.where(e_core == c)[0]
        idx_lin = np.zeros(NIDX, dtype=np.int16)
        order = np.argsort(e_chunk[sel], kind="stable")
        es = sel[order]
        mch = e_chunk[es]
        cnts = np.bincount(mch, minlength=NCHUNK)
        starts = np.concatenate([[0], np.cumsum(cnts)[:-1]])
        j_within = np.arange(len(es)) - starts[mch]
        b = j_within // P
        p = j_within % P
        lin = mch * (K * P) + b * P + p
        idx_lin[lin] = gather_row[es]
        # wrap indices: [i%16, i//16], replicated to 128 partitions
        idx16 = np.zeros((16, NIDX // 16), dtype=np.int16)
        ar = np.arange(NIDX)
        idx16[ar % 16, ar // 16] = idx_lin
        idx_tile = np.tile(idx16, (8, 1))
        # one-hot S, PRE-TRANSPOSED to [P, NBLK, P] (partition-major) so the
        # one-time resident load is a big contiguous DMA. S[p, blk, v] = 1
        # iff the edge at slot (blk, p) targets within-chunk dst slot v.
        Sp = np.zeros((P, NBLK, P), dtype=np.float32)
        Sp[p, mch * K + b, e_voff[es]] = 1.0
        Sp = Sp.astype(ml_dtypes.bfloat16)

        pm = perm[c]
        valid = pm >= 0
        xT = np.zeros((x.shape[1], PADN), dtype=ml_dtypes.bfloat16)
        xT[:, valid] = x[pm[valid]].T.astype(ml_dtypes.bfloat16)
        dv = np.zeros(PADN, dtype=np.float32)
        dv[valid] = dinv[pm[valid]]
        dinv_b = np.tile(dv[None, :], (P, 1)).astype(np.float32)
        dinv_nm = dv.reshape(NCHUNK, P).T.copy().astype(np.float32)
        per_core.append(dict(idx=idx_tile, S=Sp, xT=xT, dinv_b=dinv_b,
                             dinv_nm=dinv_nm))

    meta = dict(N=N, NPC=NPC, NCHUNK=NCHUNK, PADN=PADN, K=K, NBLK=NBLK,
                NIDX=NIDX, FIN=x.shape[1])
    return meta, per_core


# ----------------------------------------------------------------------------
# device program
# ----------------------------------------------------------------------------

_BUILD_CACHE = {}


def dma_gather_narrow(nc, out_ap, in_ap, idxs_ap, num_idxs, elem_size,
                      elem_step, queue_num, single_packet=False,
                      prepare_only=False, sem=None):
    """dma_gather with elem_size_bytes < 256 (row pitch stays 256B).

    Mirrors BassGpSimd.dma_gather (non-transpose, HBM source) without the
    conservative elem%256 assert: the ucode only requires the row STRIDE to
    be a 256B multiple (stride_bytes_256 field); elem bytes per row is free.
    """
    eng = nc.gpsimd
    assert in_ap.ap[0][0] == elem_step
    stride_bytes = elem_step * mybir.dt.size(in_ap.dtype)
    stride_bytes_256 = exact_div(stride_bytes, 256)
    _in_ap = eng.lower_ap_dma(in_ap, for_custom_bir_dma=True)
    _idxs_ap = eng.lower_ap(idxs_ap)
    _out_ap = eng.lower_ap(out_ap)
    inst = eng.add_instruction(
        mybir.InstDMAGatherAnt(
            name=eng.bass.get_next_instruction_name(),
            ins=[*_in_ap, _idxs_ap,
                 eng.lower_val_access(eng.to_reg(num_idxs))],
            outs=[_out_ap],
            transpose=False,
            num_idxs=num_idxs,
            elem_size=elem_size,
            stride_bytes_256=stride_bytes_256,
            gen_mode=int(prepare_only),
            single_packet=single_packet,
            queue_num=queue_num,
            sbuf_tokens_per_rank=0,
            sbuf_free_dim_per_rank=0,
            sbuf_free_dim_pad_per_rank=0,
            sbuf_byte_offset=0,
        )
    )
    if prepare_only:
        assert sem is not None
        inst.then_inc(sem, 16)
        return eng._track_prepare_only(inst, queue_num)
    return inst


def build_nc(meta):
    key = (tuple(sorted(meta.items())), os.environ.get("KSTOP", "full"),
           os.environ.get("KNQ", "4"), os.environ.get("KGB", "8"))
    if key in _BUILD_CACHE:
        return _BUILD_CACHE[key]

    N = meta["N"]; PADN = meta["PADN"]; NCHUNK = meta["NCHUNK"]
    K = meta["K"]; NBLK = meta["NBLK"]; NIDX = meta["NIDX"]
    TBL_ROWS = NCORES * PADN

    fins = DIMS[:-1]
    fouts = DIMS[1:]

    nc = bacc.Bacc("TRN2", target_bir_lowering=False, debug=False,
                   num_devices=NCORES, num_swdge_queues=4)

    # ---- inputs -----------------------------------------------------------
    xT_in = nc.dram_tensor("xT", [DIMS[0], PADN], BF16, kind="ExternalInput")
    dinv_in = nc.dram_tensor("dinv_b", [P, PADN], F32, kind="ExternalInput")
    S_in = nc.dram_tensor("S", [P, NBLK * P], BF16, kind="ExternalInput")
    idx_in = nc.dram_tensor("idx", [P, NIDX // 16], I16, kind="ExternalInput")
    mask_in = nc.dram_tensor("mask_b", [P, PADN], BF16, kind="ExternalInput")
    id_in = nc.dram_tensor("ident", [P, P], BF16, kind="ExternalInput")
    dinvnm_in = nc.dram_tensor("dinv_nm", [P, NCHUNK], F32, kind="ExternalInput")
    W_in, prm_in = [], []
    for l in range(NLAYERS):
        W_in.append(nc.dram_tensor(f"w{l}", [fins[l], fouts[l]], BF16,
                                   kind="ExternalInput"))
        nf = _ceil_div(fouts[l], P)
        prm_in.append(nc.dram_tensor(f"prm{l}", [P, 4 * nf], F32,
                                     kind="ExternalInput"))
    wc1_in = nc.dram_tensor("wc1", [3 * DIMS[-1], DIMS[-1]], F32, kind="ExternalInput")
    cls_in = nc.dram_tensor("cls", [1, 3 * DIMS[-1]], F32, kind="ExternalInput")
    wc2_in = nc.dram_tensor("wc2", [DIMS[-1], 2], F32, kind="ExternalInput")
    bc2_in = nc.dram_tensor("bc2", [1, 2], F32, kind="ExternalInput")
    out_t = nc.dram_tensor("out", [1, 2], F32, kind="ExternalOutput")

    gsems = [nc.alloc_semaphore(name=f"gds{q}") for q in range(4)]
    with tile.TileContext(nc) as tc:
        with (
            tc.tile_pool(name="const", bufs=1) as cp,
            tc.tile_pool(name="state", bufs=1) as sp,
            tc.tile_pool(name="work", bufs=3) as wp,
            tc.tile_pool(name="small", bufs=3) as mp,
            tc.tile_pool(name="psum", bufs=2, space="PSUM") as pp,
            tc.tile_pool(name="dram", bufs=1, space="DRAM") as dp,
        ):
            lib_inst = nc.gpsimd.load_library(library_config.mlp)

            # ---- resident constants --------------------------------------
            # critical path first: xT + W0 + dinv_nm gate the L0 linear;
            # S + idx gate the first gather (~50us in). Split across the
            # sync (SP) and scalar (Activation) HWDGE queues.
            xT = []
            for fi in range(_ceil_div(DIMS[0], P)):
                t = sp.tile([P, PADN], BF16, name=f"xT0_{fi}")
                nc.sync.dma_start(t[:], xT_in[fi * P:(fi + 1) * P, :])
                xT.append(t)
            dinv_nm = cp.tile([P, NCHUNK], F32)
            nc.scalar.dma_start(dinv_nm[:], dinvnm_in[:])

            Wsb, prmsb = [], []
            for l in range(NLAYERS):
                fin, fout = fins[l], fouts[l]
                nfi = _ceil_div(fin, P)
                tiles = []
                for fi in range(nfi):
                    w = cp.tile([min(P, fin - fi * P), fout], BF16,
                                name=f"W{l}_{fi}")
                    nc.scalar.dma_start(w[:], W_in[l][fi * P:fi * P + w.shape[0], :])
                    tiles.append(w)
                Wsb.append(tiles)
                nf = _ceil_div(fout, P)
                pt = cp.tile([P, 4 * nf], F32, name=f"prm{l}sb")
                nc.scalar.dma_start(pt[:], prm_in[l][:])
                prmsb.append(pt)
            id128 = cp.tile([P, P], BF16)
            nc.scalar.dma_start(id128[:], id_in[:])

            # idx gates the first gather: load right after xT on sync
            idxt = cp.tile([P, NIDX // 16], I16)
            nc.sync.dma_start(idxt[:], idx_in[:])
            # resident one-hot S table, loaded once (split across 2 queues);
            # only needed by the first agg matmul (~60us in)
            # keep the sync queue free after xT+idx: the L0 cc_in writes
            # (which gate the first AllGather) must not queue behind bulk
            # resident loads, so S/dinv_b/mask_b all go on scalar.
            S_res = cp.tile([P, NBLK, P], BF16, name="S_res")
            Sflat = S_res[:].rearrange("p b v -> p (b v)")
            half = (NBLK // 2) * P
            nc.scalar.dma_start(Sflat[:, :half], S_in[:, :half])
            nc.scalar.dma_start(Sflat[:, half:], S_in[:, half:])
            dinv_b = cp.tile([P, PADN], F32)
            nc.scalar.dma_start(dinv_b[:], dinv_in[:])
            mask_b = cp.tile([P, PADN], BF16)
            nc.scalar.dma_start(mask_b[:], mask_in[:])
            wc1a = cp.tile([P, DIMS[-1]], F32)
            nc.scalar.dma_start(wc1a[:], wc1_in[0:P, :])
            wc1b = cp.tile([3 * DIMS[-1] - P, DIMS[-1]], F32)
            nc.scalar.dma_start(wc1b[:], wc1_in[P:, :])
            clssb = cp.tile([1, 3 * DIMS[-1]], F32)
            nc.scalar.dma_start(clssb[:], cls_in[:])
            wc2sb = cp.tile([DIMS[-1], 2], F32)
            nc.scalar.dma_start(wc2sb[:], wc2_in[:])
            bc2sb = cp.tile([1, 2], F32)
            nc.scalar.dma_start(bc2sb[:], bc2_in[:])
            ones1 = cp.tile([1, 1], F32)
            nc.vector.memset(ones1[:], 1.0)

            inv_n = 1.0 / float(N)
            KSTOP = os.environ.get("KSTOP", "full")
            stopped = False

            # ---- layers ---------------------------------------------------
            for l in range(NLAYERS):
                if stopped:
                    break
                fin, fout = fins[l], fouts[l]
                nfi = _ceil_div(fin, P)
                nf = _ceil_div(fout, P)
                fop_last = fout - (nf - 1) * P
                elem = ELEM[l]

                tT = [sp.tile([P, PADN], F32, name=f"tT{l}_{fo}", tag=f"tT_{fo}")
                      for fo in range(nf)]

                cc_in = dp.tile([PADN, PITCH], FP8, name=f"ccin{l}",
                                tag=f"ccin_{l}")
                u_full = dp.tile([TBL_ROWS, PITCH], FP8, addr_space="Shared",
                                 name=f"ufull{l}", tag=f"ufull_{l}")
                unmq = sp.tile([P, NCHUNK, PITCH], FP8, name=f"unmq{l}",
                               tag="unmq")
                if fout < PITCH:
                    nc.vector.memset(unmq[:, :, fout:], 0.0)

                # -- linear (node-major): u[chunk] = dinv * (x[chunk] @ W) --
                with nc.named_scope(f"L{l}_linear"):
                    for m in range(NCHUNK):
                        ps = pp.tile([P, fout], F32, tag="lin",
                                     name=f"pslin{l}_{m}")
                        for fi in range(nfi):
                            nc.tensor.matmul(
                                ps[:],
                                lhsT=xT[fi][:, m * P:(m + 1) * P],
                                rhs=Wsb[l][fi][:],
                                start=(fi == 0), stop=(fi == nfi - 1))
                        nc.vector.tensor_scalar_mul(
                            unmq[:, m, :fout], ps[:], dinv_nm[:, m:m + 1])
                        nc.sync.dma_start(cc_in[m * P:(m + 1) * P, :],
                                          unmq[:, m, :])

                # -- allgather u ------------------------------------------
                with nc.named_scope(f"L{l}_ag"):
                    nc.gpsimd.collective_compute(
                        "AllGather", mybir.AluOpType.bypass,
                        replica_groups=[list(range(NCORES))],
                        ins=[cc_in.opt()], outs=[u_full.opt()])

                if KSTOP == f"ag{l}":
                    stopped = True
                    continue
                # -- gather + aggregate + combine per chunk ----------------
                sums = mp.tile([P, nf * NCHUNK], F32, tag="sums",
                               name=f"sums{l}", bufs=1)
                sums_sq = mp.tile([P, nf * NCHUNK], F32, tag="sums_sq",
                                  name=f"sumsq{l}", bufs=1)
                with nc.named_scope(f"L{l}_agg"):
                    G = int(os.environ.get("KGG", "1"))
                    NQ = int(os.environ.get("KNQ", "4"))
                    KSP = os.environ.get("KSP", "0") == "1"
                    KPT = os.environ.get("KPT", "0") == "1"
                    # two half-chunk gathers per chunk (K/2 blocks = 1024
                    # descs each) on different queues: fits the SWDGE ring
                    # 4-deep per queue so every issue fast-returns and all
                    # 4 queue drains overlap.
                    NH = 2 if (K % 2 == 0 and G == 1) else 1
                    KH = K // NH
                    gt = None
                    for m in range(NCHUNK):
                        if m % G == 0:
                            g0 = m // G
                            gsz = min(G, NCHUNK - g0 * G)
                            halves = []
                            for hh in range(NH):
                                gth = wp.tile([P, gsz * KH, elem], FP8,
                                              tag=f"gath{hh}",
                                              name=f"gt{l}_{g0}_{hh}",
                                              bufs=int(os.environ.get("KGB", "8")))
                                i0 = (g0 * G * K + hh * KH) * (P // 16)
                                i1 = i0 + gsz * KH * P // 16
                                nidx = gsz * KH * P
                                q = (NH * g0 + hh) % NQ
                                if elem >= 256:
                                    gi = nc.gpsimd.dma_gather(
                                        out_ap=gth[:], in_ap=u_full[:],
                                        idxs_ap=idxt[:, i0:i1],
                                        num_idxs=nidx, num_idxs_reg=nidx,
                                        elem_size=elem, single_packet=KSP,
                                        queue_num=q, prepare_only=KPT,
                                        sem=gsems[q] if KPT else None)
                                else:
                                    gi = dma_gather_narrow(
                                        nc, out_ap=gth[:],
                                        in_ap=u_full[:, :elem],
                                        idxs_ap=idxt[:, i0:i1],
                                        num_idxs=nidx, elem_size=elem,
                                        elem_step=PITCH, queue_num=q,
                                        single_packet=KSP, prepare_only=KPT,
                                        sem=gsems[q] if KPT else None)
                                if KPT:
                                    nc.gpsimd.trigger_dma(count=None,
                                                          queue_num=q)
                                bass._add_dep_helper(gi.ins, lib_inst.ins,
                                                     reason="lib before gather")
                                halves.append(gth)
                            gt = halves
                        mo = (m % G) * K
                        agg = [pp.tile([P if fo < nf - 1 else fop_last, P],
                                       F32, tag=f"agg{fo}",
                                       bufs=(3 if fo == 0 else 2),
                                       name=f"agg{l}_{m}_{fo}")
                               for fo in range(nf)]
                        for k in range(K):
                            gth = gt[k // KH]
                            bi = (m % G) * KH + (k % KH)
                            for fo in range(nf):
                                fop = P if fo < nf - 1 else fop_last
                                nc.tensor.matmul(
                                    agg[fo][:],
                                    lhsT=gth[:, bi, fo * P:fo * P + fop],
                                    rhs=S_res[:, m * K + k, :],
                                    start=(k == 0), stop=False)
                        for fo in range(nf):
                            fop = P if fo < nf - 1 else fop_last
                            nc.tensor.matmul(
                                agg[fo][:],
                                lhsT=unmq[:, m, fo * P:fo * P + fop],
                                rhs=id128[:],
                                start=False, stop=True)
                        for fo in range(nf):
                            fop = P if fo < nf - 1 else fop_last
                            msl = slice(m * P, (m + 1) * P)
                            tsl = tT[fo][:fop, msl]
                            nc.vector.tensor_tensor(
                                out=tsl, in0=agg[fo][:], in1=dinv_b[:fop, msl],
                                op=mybir.AluOpType.mult)
                            nc.vector.reduce_sum(
                                out=sums[:fop, (fo * NCHUNK + m):(fo * NCHUNK + m + 1)],
                                in_=tsl, axis=mybir.AxisListType.X)
                            scr = mp.tile([P, P], F32, tag="scr",
                                          name=f"scr{l}_{m}_{fo}")
                            nc.vector.tensor_tensor(
                                out=scr[:fop, :], in0=tsl, in1=tsl,
                                op=mybir.AluOpType.mult)
                            nc.vector.reduce_sum(
                                out=sums_sq[:fop, (fo * NCHUNK + m):(fo * NCHUNK + m + 1)],
                                in_=scr[:fop, :], axis=mybir.AxisListType.X)

                if KSTOP == f"agg{l}":
                    stopped = True
                    continue
                # -- stats allgather + affine ------------------------------
                with nc.named_scope(f"L{l}_stats"):
                    arow = P if nf > 1 else fop_last
                    stat = mp.tile([P, 2 * nf], F32, tag="stat", name=f"stat{l}")
                    for j in range(nf):
                        nc.vector.reduce_sum(
                            out=stat[:arow, j:j + 1],
                            in_=sums[:arow, j * NCHUNK:(j + 1) * NCHUNK],
                            axis=mybir.AxisListType.X)
                        nc.vector.reduce_sum(
                            out=stat[:arow, nf + j:nf + j + 1],
                            in_=sums_sq[:arow, j * NCHUNK:(j + 1) * NCHUNK],
                            axis=mybir.AxisListType.X)
                    ccs_in = dp.tile([arow, 2 * nf], F32, name=f"ccsin{l}",
                                     tag=f"ccs_{l}")
                    ccs_out = dp.tile([NCORES * arow, 2 * nf], F32,
                                      addr_space="Shared", name=f"ccsout{l}",
                                      tag=f"ccso_{l}")
                    nc.sync.dma_start(ccs_in[:arow, :], stat[:arow, :])
                    nc.gpsimd.collective_compute(
                        "AllGather", mybir.AluOpType.bypass,
                        replica_groups=[list(range(NCORES))],
                        ins=[ccs_in.opt()], outs=[ccs_out.opt()])
                    allst = mp.tile([P, NCORES * 2 * nf], F32, tag="allst",
                                    name=f"allst{l}")
                    nc.sync.dma_start(
                        allst[:arow].rearrange("p (r c) -> p r c", r=NCORES),
                        ccs_out.rearrange("(r p) c -> p r c", p=arow))
                    w_ = 2 * nf
                    for h in (4, 2, 1):
                        for r in range(h):
                            nc.vector.tensor_tensor(
                                out=allst[:arow, r * w_:(r + 1) * w_],
                                in0=allst[:arow, r * w_:(r + 1) * w_],
                                in1=allst[:arow, (r + h) * w_:(r + h + 1) * w_],
                                op=mybir.AluOpType.add)
                    prm = prmsb[l]          # cols: b | gw | gb | ga
                    aff = mp.tile([P, 6 * nf], F32, tag="aff", name=f"aff{l}")
                    et = aff[:arow, 0 * nf:1 * nf]
                    meanO = aff[:arow, 1 * nf:2 * nf]
                    cvar = aff[:arow, 2 * nf:3 * nf]
                    scl = aff[:arow, 3 * nf:4 * nf]
                    shf = aff[:arow, 4 * nf:5 * nf]
                    tmp = aff[:arow, 5 * nf:6 * nf]
                    b_ = prm[:arow, 0 * nf:1 * nf]
                    gw_ = prm[:arow, 1 * nf:2 * nf]
                    gb_ = prm[:arow, 2 * nf:3 * nf]
                    ga_ = prm[:arow, 3 * nf:4 * nf]
                    st_t = allst[:arow, 0:nf]
                    st_t2 = allst[:arow, nf:2 * nf]
                    V = nc.vector
                    OP = mybir.AluOpType
                    V.tensor_scalar_mul(et, st_t, inv_n)                 # E[t]
                    V.tensor_tensor(out=meanO, in0=et, in1=b_, op=OP.add)
                    V.tensor_tensor(out=tmp, in0=ga_, in1=meanO, op=OP.mult)
                    V.tensor_tensor(out=tmp, in0=b_, in1=tmp, op=OP.subtract)
                    V.tensor_tensor(out=cvar, in0=tmp, in1=et, op=OP.mult)
                    V.tensor_scalar_mul(cvar, cvar, 2.0)
                    V.tensor_scalar_mul(shf, st_t2, inv_n)               # E[t^2]
                    V.tensor_tensor(out=cvar, in0=cvar, in1=shf, op=OP.add)
                    V.tensor_tensor(out=scl, in0=tmp, in1=tmp, op=OP.mult)
                    V.tensor_tensor(out=cvar, in0=cvar, in1=scl, op=OP.add)
                    V.tensor_scalar_add(cvar, cvar, EPS)
                    V.reciprocal(scl, cvar)
                    nc.scalar.activation(scl, scl,
                                         mybir.ActivationFunctionType.Sqrt)
                    V.tensor_tensor(out=scl, in0=scl, in1=gw_, op=OP.mult)
                    V.tensor_tensor(out=shf, in0=scl, in1=tmp, op=OP.mult)
                    V.tensor_tensor(out=shf, in0=shf, in1=gb_, op=OP.add)

                if KSTOP == f"stats{l}":
                    stopped = True
                    continue
                # -- apply: x_next = Gelu(t*scale + shift) (+ residual) ----
                with nc.named_scope(f"L{l}_apply"):
                    xTn = []
                    for fo in range(nf):
                        fop = P if fo < nf - 1 else fop_last
                        xt_new = sp.tile([P, PADN], BF16, name=f"xT{l + 1}_{fo}",
                                         tag=f"xT{l + 1}_{fo}")
                        nc.scalar.activation(
                            xt_new[:fop, :], tT[fo][:fop, :],
                            mybir.ActivationFunctionType.Gelu,
                            bias=aff[:fop, 4 * nf + fo:4 * nf + fo + 1],
                            scale=aff[:fop, 3 * nf + fo:3 * nf + fo + 1])
                        if fin == fout:  # residual
                            nc.vector.tensor_tensor(
                                out=xt_new[:fop, :], in0=xt_new[:fop, :],
                                in1=xT[fo][:fop, :], op=mybir.AluOpType.add)
                        xTn.append(xt_new)
                    xT = xTn

            # ---- pooling + classifier ------------------------------------
            if stopped:
                zz = mp.tile([1, 2], F32, tag="fin", name="zz")
                nc.vector.memset(zz[:], 0.0)
                nc.sync.dma_start(out_t[:], zz[:])
            with nc.named_scope("pool_cls") if not stopped else _nullcm():
              if not stopped:
                h = DIMS[-1]
                x3 = xT[0]
                nc.vector.tensor_tensor(out=x3[:h, :], in0=x3[:h, :],
                                        in1=mask_b[:h, :],
                                        op=mybir.AluOpType.mult)
                pool = mp.tile([h, 2], F32, tag="pool", name="pool")
                nc.vector.reduce_sum(out=pool[:, 0:1], in_=x3[:h, :],
                                     axis=mybir.AxisListType.X)
                nc.vector.reduce_max(out=pool[:, 1:2], in_=x3[:h, :],
                                     axis=mybir.AxisListType.X)
                ccp_in = dp.tile([h, 2], F32, name="ccpin", tag="ccp")
                ccp_out = dp.tile([NCORES * h, 2], F32, addr_space="Shared",
                                  name="ccpout", tag="ccpo")
                nc.sync.dma_start(ccp_in[:], pool[:])
                nc.gpsimd.collective_compute(
                    "AllGather", mybir.AluOpType.bypass,
                    replica_groups=[list(range(NCORES))],
                    ins=[ccp_in.opt()], outs=[ccp_out.opt()])
                allp = mp.tile([h, NCORES * 2], F32, tag="allp", name="allp")
                nc.sync.dma_start(
                    allp[:].rearrange("p (r c) -> p r c", r=NCORES),
                    ccp_out.rearrange("(r p) c -> p r c", p=h))
                gsum = mp.tile([h, 3], F32, tag="gsum", name="gsum")
                nc.vector.reduce_sum(out=gsum[:, 2:3],
                                     in_=allp.rearrange("p (r c) -> p r c", c=2)[:, :, 0],
                                     axis=mybir.AxisListType.X)
                nc.vector.reduce_max(out=gsum[:, 1:2],
                                     in_=allp.rearrange("p (r c) -> p r c", c=2)[:, :, 1],
                                     axis=mybir.AxisListType.X)
                nc.vector.tensor_scalar_mul(gsum[:, 0:1], gsum[:, 2:3], inv_n)
                t1 = mp.tile([P, 1], F32, tag="t1", name="t1")
                t2 = mp.tile([h, 1], F32, tag="t2", name="t2")
                nc.sync.dma_start(t1[0:h, :], gsum[:, 0:1])
                nc.sync.dma_start(t1[h:2 * h, :], gsum[:, 1:2])
                nc.vector.tensor_copy(out=t2[:], in_=gsum[:, 2:3])
                psh = pp.tile([1, h], F32, tag="lin", name="psh")
                nc.tensor.matmul(psh[:], lhsT=t1[:], rhs=wc1a[:],
                                 start=True, stop=False)
                nc.tensor.matmul(psh[:], lhsT=t2[:], rhs=wc1b[:],
                                 start=False, stop=True)
                hv = mp.tile([1, h], F32, tag="hv", name="hv")
                nc.vector.tensor_tensor(out=hv[:], in0=psh[:],
                                        in1=clssb[:, 0:h],
                                        op=mybir.AluOpType.add)  # + bc1
                sc1 = mp.tile([1, 4], F32, tag="sc1", name="sc1")
                nc.vector.reduce_sum(out=sc1[:, 0:1], in_=hv[:],
                                     axis=mybir.AxisListType.X)
                nc.vector.tensor_scalar_mul(sc1[:, 0:1], sc1[:, 0:1], 1.0 / h)
                nc.vector.tensor_scalar(
                    out=hv[:], in0=hv[:], scalar1=sc1[:, 0:1], scalar2=None,
                    op0=mybir.AluOpType.subtract)
                scr2 = mp.tile([1, h], F32, tag="scr2", name="scr2")
                nc.vector.tensor_tensor(out=scr2[:], in0=hv[:], in1=hv[:],
                                        op=mybir.AluOpType.mult)
                nc.vector.reduce_sum(out=sc1[:, 1:2], in_=scr2[:],
                                     axis=mybir.AxisListType.X)
                nc.vector.tensor_scalar_mul(sc1[:, 1:2], sc1[:, 1:2], 1.0 / h)
                nc.vector.tensor_scalar_add(sc1[:, 1:2], sc1[:, 1:2], EPS)
                nc.vector.reciprocal(sc1[:, 2:3], sc1[:, 1:2])
                nc.scalar.activation(sc1[:, 2:3], sc1[:, 2:3],
                                     mybir.ActivationFunctionType.Sqrt)
                nc.vector.tensor_scalar(
                    out=hv[:], in0=hv[:], scalar1=sc1[:, 2:3], scalar2=None,
                    op0=mybir.AluOpType.mult)
                nc.vector.tensor_tensor(out=hv[:], in0=hv[:],
                                        in1=clssb[:, h:2 * h],
                                        op=mybir.AluOpType.mult)  # * ln_w
                nc.vector.tensor_tensor(out=hv[:], in0=hv[:],
                                        in1=clssb[:, 2 * h:3 * h],
                                        op=mybir.AluOpType.add)   # + ln_b
                nc.scalar.activation(hv[:], hv[:],
                                     mybir.ActivationFunctionType.Gelu)
                pst = pp.tile([h, 1], F32, tag="tr", name="pst", bufs=1)
                nc.tensor.matmul(pst[:], lhsT=hv[:], rhs=ones1[:],
                                 start=True, stop=True)
                hT = mp.tile([h, 1], F32, tag="hT", name="hT")
                nc.vector.tensor_copy(out=hT[:], in_=pst[:])
                psf = pp.tile([1, 2], F32, tag="lin", name="psf")
                nc.tensor.matmul(psf[:], lhsT=hT[:], rhs=wc2sb[:],
                                 start=True, stop=True)
                fin_sb = mp.tile([1, 2], F32, tag="fin", name="fin_sb")
                nc.vector.tensor_tensor(out=fin_sb[:], in0=psf[:],
                                        in1=bc2sb[:], op=mybir.AluOpType.add)
                nc.sync.dma_start(out_t[:], fin_sb[:])

    nc.compile()
    _BUILD_CACHE[key] = nc
    return nc


# ----------------------------------------------------------------------------
# entry point
# ----------------------------------------------------------------------------

def make_in_maps(inp, meta, per_core):
    P_ = P
    nfs = [_ceil_div(f, P_) for f in DIMS[1:]]
    prm = []
    for l in range(NLAYERS):
        nf = nfs[l]
        pt = np.zeros((P_, 4 * nf), dtype=np.float32)
        vals = (inp[f"b{l}"], inp[f"gn{l}_w"], inp[f"gn{l}_b"], inp[f"gn{l}_a"])
        for j, v in enumerate(vals):
            v = np.asarray(v, dtype=np.float32)
            for fo in range(nf):
                seg = v[fo * P_:(fo + 1) * P_]
                pt[:len(seg), j * nf + fo] = seg
        prm.append(pt)
    cls = np.concatenate([np.asarray(inp["bc1"], np.float32).ravel(),
                          np.asarray(inp["ln_w"], np.float32).ravel(),
                          np.asarray(inp["ln_b"], np.float32).ravel()])[None, :]
    in_maps = []
    for c in range(NCORES):
        pc = per_core[c]
        mask = (pc["dinv_b"] > 0).astype(ml_dtypes.bfloat16)
        in_maps.append({
            "xT": pc["xT"], "dinv_b": pc["dinv_b"],
            "S": pc["S"].reshape(128, -1),
            "idx": pc["idx"], "mask_b": mask,
            "ident": np.eye(P_, dtype=np.float32).astype(ml_dtypes.bfloat16),
            "dinv_nm": pc["dinv_nm"],
            "w0": np.asarray(inp["w0"], np.float32).astype(ml_dtypes.bfloat16),
            "w1": np.asarray(inp["w1"], np.float32).astype(ml_dtypes.bfloat16),
            "w2": np.asarray(inp["w2"], np.float32).astype(ml_dtypes.bfloat16),
            "prm0": prm[0], "prm1": prm[1], "prm2": prm[2],
            "wc1": np.asarray(inp["wc1"], np.float32),
            "cls": cls.astype(np.float32),
            "wc2": np.asarray(inp["wc2"], np.float32),
            "bc2": np.asarray(inp["bc2"], np.float32).reshape(1, 2),
        })
    return in_maps


def kernel(x, edge_index,
           w0, b0, gn0_w, gn0_b, gn0_a,
           w1, b1, gn1_w, gn1_b, gn1_a,
           w2, b2, gn2_w, gn2_b, gn2_a,
           wc1, bc1, ln_w, ln_b, wc2, bc2, _run_info=None):
    x = np.ascontiguousarray(np.asarray(x, dtype=np.float32))
    edge_index = np.asarray(edge_index)
    meta, per_core = preprocess(x, edge_index)
    nc = build_nc(meta)
    inp = dict(x=x, edge_index=edge_index,
               w0=w0, b0=b0, gn0_w=gn0_w, gn0_b=gn0_b, gn0_a=gn0_a,
               w1=w1, b1=b1, gn1_w=gn1_w, gn1_b=gn1_b, gn1_a=gn1_a,
               w2=w2, b2=b2, gn2_w=gn2_w, gn2_b=gn2_b, gn2_a=gn2_a,
               wc1=wc1, bc1=bc1, ln_w=ln_w, ln_b=ln_b, wc2=wc2, bc2=bc2)
    in_maps = make_in_maps(inp, meta, per_core)
    trace = bool(_run_info is not None and _run_info.get("trace"))
    res = bass_utils.run_bass_kernel_spmd(
        nc, in_maps, core_ids=list(range(NCORES)), trace=trace)
    if _run_info is not None:
        _run_info["exec_time_ns"] = res.exec_time_ns
        _run_info["scope_times"] = res.per_core_scope_times
        _run_info["insts"] = (res.instructions_and_trace or (None,))[0]
    return res.results[0]["out"]

